# revision 21
# baseline (speedup 1.0000x reference)
"""DiagGateRNN Trainium2 Bass kernel.

Problem: B=64, T=512, D=128, H=512 gated RNN with 5 outputs
(ys[B,T,1], hs/ss/leaks/rdiags [B,T,H]).

Strategy (hardcoded): data-parallel over batch B across 8 cores
(B_loc = 8 per core); all weights replicated. Everything on-chip is
H-major ([128 partitions = h-within-chunk, free = (chunk, batch)]) so
the per-step elementwise work runs on ~[128, 32] tiles.

Per core:
  Phase A: input projections gT[h, (xp|sp chunk), t, b] = Win @ x.T + b
           as one big fp32r GEMM (weights stationary), kept in SBUF.
  Phase B: T=512 fully unrolled recurrent steps; gates via
           weight-stationary matmuls accumulating in PSUM
           (out[h-chunk, b] = sum_k W.T[k,m-chunk].T @ hT[k-chunk]);
           tanh-only activations (sigmoid(x) = 0.5 + 0.5*tanh(0.5x));
           fused DVE ops for state update + rdiag; ys via 4 tiny
           matmuls against out_w per step; one packed DMA store/step.
"""

import os
import sys

sys.path.insert(0, "/opt/trn_rl_repo")

import numpy as np

import concourse.bass as bass
import concourse.bacc as bacc
import concourse.mybir as mybir
import concourse.tile as tile
from concourse.bass_utils import run_bass_kernel_spmd

B, T, D, H = 64, 512, 128, 512
NCORES = 8
BL = B // NCORES          # 8 batch per core
NCH = H // 128            # 4 h-chunks
SB = NCH * BL             # 32 = (chunk, b) free size of state tiles
F32 = mybir.dt.float32
F32R = mybir.dt.float32r
BF16 = mybir.dt.bfloat16

# dtype for the recurrent gate matmuls: "f32" exact (2-pass PE),
# "f32r" 1-pass reduced-precision multiply, "bf16" (fast weight load).
RECUR_DT = os.environ.get("RECUR_DT", "f32")
# preload gT into PSUM (ACT reads psum directly, drops the pre-add DVE op
# from the critical path; relies on walrus' dummy-matmul has_written fix)
PSUM_PRELOAD = os.environ.get("PSUM_PRELOAD", "0") == "1"
YS_CHUNK = 64
AluOp = mybir.AluOpType
Act = mybir.ActivationFunctionType


def _r(ap):
    """View an fp32 AP as float32r for fast matmul streaming."""
    return ap.bitcast(F32R)


def _id(ap):
    return ap


def build_kernel(nc: bass.Bass, t_steps: int = T):
    f32 = F32
    xT = nc.dram_tensor("xT", [128, BL * T], f32, kind="ExternalInput")
    wcat = nc.dram_tensor("wcat", [128, 2 * NCH * NCH * 128], f32, kind="ExternalInput")
    wxin = nc.dram_tensor("wxin", [128, 2 * NCH * 128], f32, kind="ExternalInput")
    bias2 = nc.dram_tensor("bias2", [128, 2 * NCH], f32, kind="ExternalInput")
    diag2 = nc.dram_tensor("diag2", [128, 2 * NCH], f32, kind="ExternalInput")
    outw = nc.dram_tensor("outw", [128, NCH], f32, kind="ExternalInput")
    outb = nc.dram_tensor("outb", [1, 1], f32, kind="ExternalInput")

    # packed per-step output: [t, p, (hs|ss|lk|rd), (chunk, b)]
    o_all = nc.dram_tensor("o_all", [t_steps, 128, 4, SB], f32, kind="ExternalOutput")
    o_ys = nc.dram_tensor("o_ys", [1, BL * t_steps], f32, kind="ExternalOutput")

    conv_dt = {"bf16": BF16, "f32r": F32R}.get(RECUR_DT)

    # Walrus allows only ONE sync-wait on LDWEIGHTS micro-ops (matmuls) and
    # on DMA instructions.  Consequences for this kernel:
    #  - every matmul SBUF operand is produced by a DVE copy ("laundering"),
    #    so matmul waits collapse onto the DVE semaphore;
    #  - DMA stores must read tiles written by a single engine.
    with tile.TileContext(nc) as tc:
        with (
            tc.tile_pool(name="big", bufs=1) as big,
            tc.tile_pool(name="consts", bufs=1) as consts,
            tc.tile_pool(name="ps_pre", bufs=2, space="PSUM") as pps_pre,
            tc.tile_pool(name="ps_rec", bufs=3, space="PSUM") as pps_rec,
            tc.tile_pool(name="psy", bufs=2, space="PSUM") as ppy,
        ):
            # ---- small constants ----
            b2_sb = consts.tile([128, 2 * NCH], f32)
            nc.gpsimd.dma_start(out=b2_sb, in_=bias2[:, :])
            d2_sb = consts.tile([128, 2 * NCH], f32)
            nc.gpsimd.dma_start(out=d2_sb, in_=diag2[:, :])
            ow_dma = consts.tile([128, NCH], f32)
            nc.gpsimd.dma_start(out=ow_dma, in_=outw[:, :])
            ow_sb = consts.tile([128, NCH], f32)
            nc.vector.tensor_copy(out=ow_sb, in_=ow_dma)
            ob_sb = consts.tile([1, 1], f32)
            nc.gpsimd.dma_start(out=ob_sb, in_=outb[:, :])

            # broadcast diagonals over b: [128, NCH] -> [128, NCH, BL]
            whd = consts.tile([128, NCH, BL], f32)
            usd = consts.tile([128, NCH, BL], f32)
            d2t = d2_sb.tensor
            whd_src = bass.AP(
                tensor=d2t, offset=d2_sb.offset,
                ap=[d2_sb.ap[0], [1, NCH], [0, BL]],
            )
            usd_src = bass.AP(
                tensor=d2t, offset=d2_sb.offset + NCH,
                ap=[d2_sb.ap[0], [1, NCH], [0, BL]],
            )
            nc.vector.tensor_copy(out=whd, in_=whd_src)
            nc.vector.tensor_copy(out=usd, in_=usd_src)
            half = consts.tile([128, 1], f32)
            nc.vector.memset(half, 0.5)

            # ---- gate weights: DMA stage + DVE copy into matmul dtype ----
            w_sb = consts.tile([128, 2 * NCH * NCH * 128], conv_dt or F32)
            pw = tc.alloc_tile_pool(name="pw", bufs=1)
            w_dma = pw.tile([128, 2 * NCH * NCH * 128], f32, tag="w_dma")
            nc.gpsimd.dma_start(out=w_dma, in_=wcat[:, :])
            nc.vector.tensor_copy(out=w_sb, in_=w_dma)
            pw.release()

            # ---- Phase A: gT[128, 2*NCH slices, T, BL] = Win @ xT + bias ----
            gT = big.tile([128, 2 * NCH, T, BL], f32, tag="gT")
            pa = tc.alloc_tile_pool(name="pa", bufs=1)
            wx_dma = pa.tile([128, 2 * NCH * 128], f32, tag="wx_dma")
            nc.gpsimd.dma_start(out=wx_dma, in_=wxin[:, :])
            wx_sb = pa.tile([128, 2 * NCH * 128], F32R, tag="wx_sb")
            nc.vector.tensor_copy(out=wx_sb, in_=wx_dma)
            pxt = tc.alloc_tile_pool(name="pxt", bufs=2)
            n_chunks = BL * T // 512  # 8 chunks of 512 (t,b)-cols; b-major: chunk == b
            for n in range(n_chunks):
                xt_dma = pxt.tile([128, 512], f32, tag="xtd")
                nc.gpsimd.dma_start(out=xt_dma, in_=xT[:, n * 512:(n + 1) * 512])
                xt_sb = pxt.tile([128, 512], F32R, tag="xts")
                nc.vector.tensor_copy(out=xt_sb, in_=xt_dma)
                for m in range(2 * NCH):
                    ps = pps_pre.tile([128, 512], f32, tag="ps_pre")
                    nc.tensor.matmul(
                        ps,
                        wx_sb[:, m * 128:(m + 1) * 128],
                        xt_sb,
                        start=True, stop=True,
                    )
                    dst = gT[:, m, :, n]  # [128, T] strided
                    if (m + n) % 2 == 0:
                        nc.vector.tensor_scalar(
                            out=dst, in0=ps,
                            scalar1=b2_sb[:, m:m + 1], scalar2=None,
                            op0=AluOp.add,
                        )
                    else:
                        nc.scalar.activation(
                            dst, ps, Act.Identity,
                            bias=b2_sb[:, m:m + 1], scale=1.0,
                        )
            pxt.release()
            pa.release()

            # ---- Phase B: recurrence ----
            work = tc.alloc_tile_pool(name="work", bufs=4)
            yspool = tc.alloc_tile_pool(name="yspool", bufs=2)
            stpool = tc.alloc_tile_pool(name="stage", bufs=4)
            hpool = tc.alloc_tile_pool(name="hpool", bufs=4)
            h0 = hpool.tile([128, SB], f32, tag="h0")
            nc.vector.memset(h0, 0.0)
            h = h0
            if conv_dt is not None:
                hr = hpool.tile([128, SB], conv_dt, tag="hr")
                nc.vector.tensor_copy(out=hr, in_=h0)

            for t in range(t_steps):
                ps = pps_rec.tile([128, 2 * SB], f32, tag="ps_rec")
                h_mm = hr if conv_dt is not None else h
                if PSUM_PRELOAD:
                    nc.vector.tensor_copy(out=ps, in_=gT[:, :, t, :])
                for g in range(2):
                    for m in range(NCH):
                        for k in range(NCH):
                            blk = (g * NCH + m) * NCH + k
                            nc.tensor.matmul(
                                ps[:, (g * NCH + m) * BL:(g * NCH + m + 1) * BL],
                                w_sb[:, blk * 128:(blk + 1) * 128],
                                h_mm[:, k * BL:(k + 1) * BL],
                                start=(k == 0 and not PSUM_PRELOAD),
                                stop=(k == NCH - 1),
                            )
                if PSUM_PRELOAD:
                    pre = ps
                else:
                    pre = work.tile([128, 2 * SB], f32, tag="pre")
                    nc.vector.tensor_add(pre, ps, gT[:, :, t, :])
                ht = work.tile([128, SB], f32, tag="ht")
                nc.scalar.activation(ht, pre[:, 0:SB], Act.Tanh)
                sg = work.tile([128, SB], f32, tag="sg")
                nc.scalar.activation(sg, pre[:, SB:2 * SB], Act.Tanh, scale=0.5)

                # stage tile: slices 0,1 DVE-written (h_new, rdiag);
                # slices 2,3 ACT-written (s, leak) -> two contiguous stores
                stage = stpool.tile([128, 4, SB], f32, tag="stage")
                h_new = stage[:, 0, :]
                rd = stage[:, 1, :]
                s_out = stage[:, 2, :]
                leak = stage[:, 3, :]

                d = work.tile([128, SB], f32, tag="d")
                nc.vector.tensor_sub(d, ht, h)
                sd2 = work.tile([128, SB], f32, tag="sd2")
                nc.vector.scalar_tensor_tensor(
                    sd2, in0=sg, scalar=1.0, in1=d, op0=AluOp.add, op1=AluOp.mult)
                nc.vector.scalar_tensor_tensor(
                    h_new, in0=sd2, scalar=0.5, in1=h, op0=AluOp.mult, op1=AluOp.add)

                # ys: psum[1, BL] = sum_s out_w_chunk.T @ h_new_chunk (+ out_b)
                py = ppy.tile([1, BL], f32, tag="py")
                for s_ in range(NCH):
                    nc.tensor.matmul(
                        py, ow_sb[:, s_:s_ + 1], h_new[:, s_ * BL:(s_ + 1) * BL],
                        start=(s_ == 0), stop=(s_ == NCH - 1),
                    )
                if t % YS_CHUNK == 0:
                    ys_acc = yspool.tile([1, YS_CHUNK * BL], f32, tag="ys_acc")
                tl = t % YS_CHUNK
                nc.vector.tensor_scalar(
                    out=ys_acc[0:1, tl * BL:(tl + 1) * BL], in0=py,
                    scalar1=ob_sb[0:1, 0:1], scalar2=None, op0=AluOp.add)
                if t % YS_CHUNK == YS_CHUNK - 1 or t == t_steps - 1:
                    t0c = (t // YS_CHUNK) * YS_CHUNK
                    nc.sync.dma_start(
                        out=o_ys[0:1, t0c * BL:(t + 1) * BL],
                        in_=ys_acc[0:1, 0:(t + 1 - t0c) * BL])

                # gate outputs (ACT)
                nc.scalar.activation(s_out, sg, Act.Identity, bias=half[:, 0:1], scale=0.5)
                nc.scalar.activation(leak, sg, Act.Identity, bias=half[:, 0:1], scale=-0.5)

                # rdiag = d*(s*leak*usd) + s*(1-ht^2)*whd   (DVE)
                q = work.tile([128, SB], f32, tag="q")
                nc.vector.tensor_mul(q, ht, ht)
                rb1 = work.tile([128, SB], f32, tag="rb1")
                nc.vector.scalar_tensor_tensor(
                    rb1, in0=q, scalar=-1.0, in1=whd, op0=AluOp.add, op1=AluOp.mult)
                rb2 = work.tile([128, SB], f32, tag="rb2")
                nc.vector.scalar_tensor_tensor(
                    rb2, in0=sg, scalar=1.0, in1=rb1, op0=AluOp.add, op1=AluOp.mult)
                ra0 = work.tile([128, SB], f32, tag="ra0")
                nc.vector.tensor_mul(ra0, s_out, leak)
                ra1 = work.tile([128, SB], f32, tag="ra1")
                nc.vector.tensor_mul(ra1, ra0, usd)
                ra2 = work.tile([128, SB], f32, tag="ra2")
                nc.vector.tensor_mul(ra2, ra1, d)
                nc.vector.scalar_tensor_tensor(
                    rd, in0=rb2, scalar=-0.5, in1=ra2, op0=AluOp.mult, op1=AluOp.add)

                # split stores by producer engine (1-wait DMA limit):
                # DVE wrote slices 0 and 3, ACT wrote slices 1 and 2
                nc.sync.dma_start(
                    out=o_all[t, :, 0:2, :], in_=stage[:, 0:2, :])
                nc.sync.dma_start(
                    out=o_all[t, :, 2:4, :], in_=stage[:, 2:4, :])

                h = h_new
                if conv_dt is not None:
                    hr = hpool.tile([128, SB], conv_dt, tag="hr")
                    nc.vector.tensor_copy(out=hr, in_=h_new)

            for p in (hpool, stpool, yspool, work):
                p.release()
    return nc


def _prep_inputs(x, Wx_w, Wx_b, Wh_w, Ws_w, Ws_b, Us_w, out_w, out_b):
    """Host-side layout prep (transposes/concats only, no math)."""
    f = np.float32
    x = np.asarray(x, f)
    wcat = np.empty((128, 2 * NCH * NCH * 128), f)
    for g, W in ((0, np.asarray(Wh_w, f)), (1, np.asarray(Us_w, f))):
        for m in range(NCH):
            for k in range(NCH):
                blk = (g * NCH + m) * NCH + k
                wcat[:, blk * 128:(blk + 1) * 128] = \
                    W[m * 128:(m + 1) * 128, k * 128:(k + 1) * 128].T
    wxin = np.empty((128, 2 * NCH * 128), f)
    for gp, W in ((0, np.asarray(Wx_w, f)), (1, np.asarray(Ws_w, f))):
        for m in range(NCH):
            wxin[:, (gp * NCH + m) * 128:(gp * NCH + m + 1) * 128] = \
                W[m * 128:(m + 1) * 128, :].T
    bias2 = np.empty((128, 2 * NCH), f)
    bias2[:, 0:NCH] = np.asarray(Wx_b, f).reshape(NCH, 128).T
    bias2[:, NCH:] = np.asarray(Ws_b, f).reshape(NCH, 128).T
    diag2 = np.empty((128, 2 * NCH), f)
    diag2[:, 0:NCH] = np.diagonal(np.asarray(Wh_w, f)).reshape(NCH, 128).T
    diag2[:, NCH:] = np.diagonal(np.asarray(Us_w, f)).reshape(NCH, 128).T
    outw = np.asarray(out_w, f).reshape(NCH, 128).T.copy()
    outb = np.asarray(out_b, f).reshape(1, 1)

    shared = dict(wcat=wcat, wxin=wxin, bias2=bias2, diag2=diag2,
                  outw=outw, outb=outb)
    in_maps = []
    for c in range(NCORES):
        xs = x[c * BL:(c + 1) * BL]                    # [BL, T, D]
        xT = np.ascontiguousarray(xs.transpose(2, 0, 1).reshape(128, BL * T))
        m = dict(shared)
        m["xT"] = xT
        in_maps.append(m)
    return in_maps


def make_nc(t_steps=T):
    nc = build_kernel(bacc.Bacc("TRN2"), t_steps=t_steps)
    nc.compile()
    return nc


def kernel(**inputs):
    in_maps = _prep_inputs(**inputs)
    nc = make_nc()
    res = run_bass_kernel_spmd(nc, in_maps, core_ids=list(range(NCORES)))
    return _assemble(res.results)


def _assemble(results, t_steps=T):
    f = np.float32
    ys = np.empty((B, t_steps, 1), f)
    outs = [np.empty((B, t_steps, H), f) for _ in range(4)]  # hs, ss, lk, rd
    for c, out in enumerate(results):
        bs = slice(c * BL, (c + 1) * BL)
        a = np.asarray(out["o_all"])  # [T, 128, 4, SB]; slices: hs, rd, ss, lk
        a = (a.reshape(t_steps, 128, 4, NCH, BL)
             .transpose(2, 4, 0, 3, 1)  # [4, b, t, chunk, p]
             .reshape(4, BL, t_steps, H))
        for j, dst in enumerate((0, 3, 1, 2)):
            outs[dst][bs] = a[j]
        ys[bs] = np.asarray(out["o_ys"]).reshape(t_steps, BL).T.reshape(BL, t_steps, 1)
    hs, ss, lk, rd = outs
    return ys, hs, ss, lk, rd
